# revision 21
# baseline (speedup 1.0000x reference)
"""Trainium2 Bass kernel for a 12-layer tied-weight dense transformer + CE loss.

Sharding: 8 cores = 2 batch groups x 4-way sequence shard (224 rows/core).
All big GEMMs run in bf16 with fp32 PSUM accumulation; residual stream,
layernorm and softmax statistics are fp32.  Activations live transposed
([D on partitions, rows on free]) so every GEMM contracts on the partition
axis.  Per layer the only collective is one AllGather of k/v within each
4-core group; the loss does one tiny 8-core AllReduce at the end.
"""

import os
import sys

sys.path.insert(0, "/opt/trn_rl_repo")

import numpy as np
import ml_dtypes

import concourse.bass as bass
import concourse.tile as tile
from concourse import bacc, mybir
from concourse.bass_utils import run_bass_kernel_spmd


def _install_ntff_hook_shim():
    """The agent image's antenv lacks axon_hooks; synthesize it so
    run_bass_kernel_spmd(trace=True) can NTFF-profile via libaxon_pjrt."""
    import types
    try:
        import antenv.axon_hooks  # noqa: F401
        return
    except ImportError:
        pass
    mod = types.ModuleType("antenv.axon_hooks")
    _h = [None]
    mod.get_axon_ntff_profile_hook = lambda: _h[0]
    mod.set_axon_ntff_profile_hook = lambda hook: _h.__setitem__(0, hook)
    sys.modules["antenv.axon_hooks"] = mod
    try:
        from trn_agent_boot.trn_boot import _ntff_profile_via_ctypes
        hook = _ntff_profile_via_ctypes("/opt/axon/libaxon_pjrt.so")
        if hook is not None:
            mod.set_axon_ntff_profile_hook(hook)
    except Exception:
        pass


_install_ntff_hook_shim()

F32 = mybir.dt.float32
BF16 = mybir.dt.bfloat16
AF = mybir.ActivationFunctionType
ALU = mybir.AluOpType
AX = mybir.AxisListType

# Problem constants (hardcoded per contract)
B = 2
T = 896
D = 1024
H = 16
DH = 64
FF = 4096
V = 1024
P = 128
KT = D // P            # 8 k-tiles over D
R = 224                # rows per core (T * B / 8)
NG = 4                 # cores per group (sequence shard)
KC = 8                 # key chunks of 112 (112*8 == T, and 112 divides 224)
KW = 112               # key chunk width
EPS = 1e-5
NL_DEFAULT = 12

KSZ = D * R            # elems in k.T block of kv bounce (229376)

LAST_EXEC_NS = None
LAST_RESULTS = None


def _bf(x):
    return np.ascontiguousarray(np.asarray(x, np.float32).astype(ml_dtypes.bfloat16))


def _f32(x):
    return np.ascontiguousarray(np.asarray(x, np.float32))


def prepare_inputs(text, prom, code, mask, gt, gt_mask, Wqkv, bqkv, Wo, bo,
                   W1, b1, W2, b2, g1, beta1, g2, beta2, wave_emb_w):
    """Host-side shard/layout prep. Returns list of 8 per-core input dicts."""
    x = np.concatenate([_f32(text), _f32(prom), _f32(code)], axis=1)  # (2,896,1024)
    tc_ = code.shape[1]
    m = _f32(mask).copy()
    m[:, :tc_, -tc_:] = 0.0
    m[:, -tc_:, -tc_:] = np.tril(m[:, -tc_:, -tc_:])
    emask = np.exp(m)  # additive 0/1 mask -> multiplicative exp factor
    gt = np.asarray(gt).astype(np.int64)
    gtm = np.asarray(gt_mask).astype(bool)

    Wqkv = _f32(Wqkv); Wo = _f32(Wo); W1 = _f32(W1); W2 = _f32(W2)
    emb = _f32(wave_emb_w)

    # shared weight layouts (identical on every core)
    wqkvR = _bf(Wqkv.reshape(3 * KT, P, KT, P).transpose(0, 3, 2, 1).reshape(3 * KT, P, D))  # [24,128,1024] (m,p,(k j))
    wvK = _bf(Wqkv[2 * D:].T.reshape(KT, P, D))                          # [8,128,1024] (k,p,f)
    w1R = _bf(W1.reshape(FF // P, P, KT, P).transpose(0, 3, 2, 1).reshape(FF // P, P, D))  # [32,128,1024]
    w2S = _bf(W2.T.reshape(FF // P, P, D).transpose(1, 0, 2))            # [128,32,1024]
    woS = _bf(Wo.T.reshape(KT, P, D).transpose(1, 0, 2))                 # [128,8,1024]
    embR = _bf(emb.reshape(V // P, P, KT, P).transpose(0, 3, 2, 1).reshape(V // P, P, D))  # [8,128,1024]

    bqkvP = _f32(np.asarray(bqkv, np.float32).reshape(3 * KT, P).T)      # [128,24]
    bqkvQ = _f32(np.asarray(bqkv, np.float32)[:D].reshape(H, DH).T)      # [64,16]
    bvF = _f32(np.asarray(bqkv, np.float32)[2 * D:].reshape(1, D))       # [1,1024]
    b1P = _f32(np.asarray(b1, np.float32).reshape(FF // P, P).T)         # [128,32]
    b2P = _f32(np.asarray(b2, np.float32).reshape(KT, P).T)              # [128,8]
    boP = _f32(np.asarray(bo, np.float32).reshape(KT, P).T)              # [128,8]
    g1P = _f32(np.asarray(g1, np.float32).reshape(KT, P).T)
    be1P = _f32(np.asarray(beta1, np.float32).reshape(KT, P).T)
    g2P = _f32(np.asarray(g2, np.float32).reshape(KT, P).T)
    be2P = _f32(np.asarray(beta2, np.float32).reshape(KT, P).T)

    shared = dict(wqkvR=wqkvR, wvK=wvK, w1R=w1R, w2S=w2S, woS=woS, embR=embR,
                  bqkvP=bqkvP, bqkvQ=bqkvQ, bvF=bvF, b1P=b1P, b2P=b2P, boP=boP,
                  g1P=g1P, be1P=be1P, g2P=g2P, be2P=be2P)

    in_maps = []
    Lg = gt.shape[1]
    lo = T - Lg - 1            # first predicted position (383)
    hi = T - 2                 # last predicted position (894)
    for core in range(8):
        b = core // NG
        c = core % NG
        rows = slice(c * R, (c + 1) * R)
        xT0 = _f32(x[b, rows].T.reshape(KT, P, R).transpose(1, 0, 2))    # [128,8,224]
        emaskT = _bf(emask[b, rows].T.reshape(KC, KW, R).transpose(1, 0, 2))  # [112,8,224]
        oh = np.zeros((V, R), np.float32)
        wv = np.zeros((1, R), np.float32)
        for r in range(R):
            pos = c * R + r
            if lo <= pos <= hi:
                j = pos - lo
                oh[int(gt[b, j]), r] = 1.0
                wv[0, r] = 1.0 if gtm[b, j] else 0.0
        onehotT = _bf(oh.reshape(KT, P, R).transpose(1, 0, 2))           # [128,8,224]
        d = dict(shared)
        d.update(xT0=xT0, emaskT=emaskT, onehotT=onehotT, wvalid=wv)
        in_maps.append(d)
    return in_maps


# --------------------------------------------------------------------------
# device program
# --------------------------------------------------------------------------

_PROG_CACHE = {}


def build_program(n_layers=NL_DEFAULT):
    if n_layers in _PROG_CACHE:
        return _PROG_CACHE[n_layers]
    nc = bacc.Bacc("TRN2", target_bir_lowering=False, debug=False,
                   enable_asserts=False, num_devices=8)

    def din(name, shape, dt):
        return nc.dram_tensor(name, list(shape), dt, kind="ExternalInput").ap()

    xT0 = din("xT0", [P, KT, R], F32)
    wqkvR = din("wqkvR", [3 * KT, P, KT * P], BF16)
    wvK = din("wvK", [KT, P, D], BF16)
    w1R = din("w1R", [FF // P, P, KT * P], BF16)
    w2S = din("w2S", [P, FF // P, D], BF16)
    woS = din("woS", [P, KT, D], BF16)
    embR = din("embR", [V // P, P, KT * P], BF16)
    bqkvP = din("bqkvP", [P, 3 * KT], F32)
    bqkvQ = din("bqkvQ", [DH, H], F32)
    bvF = din("bvF", [1, D], F32)
    b1P = din("b1P", [P, FF // P], F32)
    b2P = din("b2P", [P, KT], F32)
    boP = din("boP", [P, KT], F32)
    g1P = din("g1P", [P, KT], F32)
    be1P = din("be1P", [P, KT], F32)
    g2P = din("g2P", [P, KT], F32)
    be2P = din("be2P", [P, KT], F32)
    emaskT = din("emaskT", [KW, KC, R], BF16)
    onehotT = din("onehotT", [P, KT, R], BF16)
    wvalid = din("wvalid", [1, R], F32)

    loss_out = nc.dram_tensor("loss", [1, 1], F32, kind="ExternalOutput").ap()

    groups4 = [[0, 1, 2, 3], [4, 5, 6, 7]]
    groups8 = [[0, 1, 2, 3, 4, 5, 6, 7]]

    with tile.TileContext(nc) as tc:
        from contextlib import ExitStack
        with ExitStack() as ctx:
            sing = ctx.enter_context(tc.tile_pool(name="sing", bufs=1))
            wq_pool = ctx.enter_context(tc.tile_pool(name="wq", bufs=3))
            wv_pool = ctx.enter_context(tc.tile_pool(name="wv", bufs=3))
            w1_pool = ctx.enter_context(tc.tile_pool(name="w1", bufs=3))
            kh_pool = ctx.enter_context(tc.tile_pool(name="kh", bufs=3))
            vp_pool = ctx.enter_context(tc.tile_pool(name="vp", bufs=8))
            pt_pool = ctx.enter_context(tc.tile_pool(name="pt", bufs=4))
            bt_pool = ctx.enter_context(tc.tile_pool(name="bt", bufs=3))   # kv bounce tiles
            tmp_pool = ctx.enter_context(tc.tile_pool(name="tmp", bufs=2))
            st_pool = ctx.enter_context(tc.tile_pool(name="st", bufs=1))
            at_pool = ctx.enter_context(tc.tile_pool(name="at", bufs=3))
            sb_pool = ctx.enter_context(tc.tile_pool(name="sb", bufs=3))   # softmax bcast sbuf
            psA = ctx.enter_context(tc.tile_pool(name="psA", bufs=2, space="PSUM"))
            psV = ctx.enter_context(tc.tile_pool(name="psV", bufs=2, space="PSUM"))
            psO = ctx.enter_context(tc.tile_pool(name="psO", bufs=1, space="PSUM"))
            psB = ctx.enter_context(tc.tile_pool(name="psB", bufs=1, space="PSUM"))
            psT = ctx.enter_context(tc.tile_pool(name="psT", bufs=1, space="PSUM"))  # stats [1,x]
            dram = ctx.enter_context(tc.tile_pool(name="dram", bufs=2, space="DRAM"))
            dram1 = ctx.enter_context(tc.tile_pool(name="dram1", bufs=1, space="DRAM"))

            # ---------------- persistent SBUF state ----------------
            xm = sing.tile([P, KT, R], F32)        # residual stream, transposed
            xb = sing.tile([P, KT, R], BF16)       # bf16 shadow of xm
            qT2 = sing.tile([DH, H, R], BF16)      # q per head
            oT = sing.tile([P, KT, R], BF16)       # attention out (heads packed 2/tile)
            w2s = sing.tile([P, FF // P, D], BF16)
            wos = sing.tile([P, KT, D], BF16)
            h32 = sing.tile([P, FF // P, R], BF16)
            emk = sing.tile([KW, KC, R], BF16)
            ohs = sing.tile([P, KT, R], BF16)
            wvs = sing.tile([1, R], F32)
            bqkvs = sing.tile([P, 3 * KT], F32)
            bqkvq = sing.tile([DH, H], F32)
            bvFs = sing.tile([KW, D], BF16)        # v bias broadcast across partitions
            b1s = sing.tile([P, FF // P], F32)
            b2s = sing.tile([P, KT], F32)
            bos = sing.tile([P, KT], F32)
            g1s = sing.tile([P, KT], F32)
            be1s = sing.tile([P, KT], F32)
            g2s = sing.tile([P, KT], F32)
            be2s = sing.tile([P, KT], F32)
            ones_bf = sing.tile([P, 1], BF16)
            ones_f = sing.tile([P, 1], F32)
            ones1_bf = sing.tile([1, DH], BF16)    # K=1 lhsT for [1,*]->[64,*] bcast
            ones1_f = sing.tile([1, P], F32)       # K=1 lhsT for [1,*]->[128,*] bcast
            epst = sing.tile([1, 1], F32)
            nc.vector.memset(epst[:], EPS)

            nc.vector.memset(ones_bf[:], 1.0)
            nc.vector.memset(ones_f[:], 1.0)
            nc.vector.memset(ones1_bf[:], 1.0)
            nc.vector.memset(ones1_f[:], 1.0)

            nc.sync.dma_start(xm[:], xT0[:])
            nc.sync.dma_start(w2s[:], w2S[:])
            nc.sync.dma_start(wos[:], woS[:])
            nc.sync.dma_start(emk[:], emaskT[:])
            nc.sync.dma_start(ohs[:], onehotT[:])
            nc.sync.dma_start(wvs[:], wvalid[:])
            nc.sync.dma_start(bqkvs[:], bqkvP[:])
            nc.sync.dma_start(bqkvq[:], bqkvQ[:])
            nc.sync.dma_start(b1s[:], b1P[:])
            nc.sync.dma_start(b2s[:], b2P[:])
            nc.sync.dma_start(bos[:], boP[:])
            nc.sync.dma_start(g1s[:], g1P[:])
            nc.sync.dma_start(be1s[:], be1P[:])
            nc.sync.dma_start(g2s[:], g2P[:])
            nc.sync.dma_start(be2s[:], be2P[:])

            # broadcast v-bias [1,1024] -> [112,1024] once (via K=1 matmul)
            bv1 = sing.tile([1, D], F32)
            nc.sync.dma_start(bv1[:], bvF[:])
            for half in range(2):
                ps = psV.tile([KW, 512], F32, name="bvbc", tag="psv")
                nc.tensor.matmul(ps[:], ones1_f[:, :KW], bv1[:, half * 512:(half + 1) * 512],
                                 start=True, stop=True)
                nc.vector.tensor_copy(out=bvFs[:, half * 512:(half + 1) * 512], in_=ps[:])

            # initial bf16 shadow
            for k in range(KT):
                nc.vector.tensor_copy(out=xb[:, k], in_=xm[:, k])

            def layernorm(gs, bes):
                # pre-LN bf16 copy for stats
                for k in range(KT):
                    nc.vector.tensor_copy(out=xb[:, k], in_=xm[:, k])
                ps_s = psT.tile([1, R], F32, name="ln_s", tag="stat_a")
                ps_q = psT.tile([1, R], F32, name="ln_q", tag="stat_b")
                for k in range(KT):
                    nc.tensor.matmul(ps_s[:], ones_bf[:], xb[:, k],
                                     start=(k == 0), stop=(k == KT - 1))
                    sq = tmp_pool.tile([P, R], BF16, tag="ln_sq")
                    nc.vector.tensor_mul(sq[:], xb[:, k], xb[:, k])
                    nc.tensor.matmul(ps_q[:], ones_bf[:], sq[:],
                                     start=(k == 0), stop=(k == KT - 1))
                mu = st_pool.tile([1, R], F32, tag="ln_mu")
                m2 = st_pool.tile([1, R], F32, tag="ln_m2")
                nc.vector.tensor_scalar_mul(mu[:], ps_s[:], 1.0 / D)
                nc.vector.tensor_scalar_mul(m2[:], ps_q[:], 1.0 / D)
                mu2 = st_pool.tile([1, R], F32, tag="ln_mu2")
                nc.vector.tensor_mul(mu2[:], mu[:], mu[:])
                st2 = st_pool.tile([1, 2, R], F32, tag="ln_st2")
                var = st_pool.tile([1, R], F32, tag="ln_var")
                nc.vector.tensor_sub(var[:], m2[:], mu2[:])
                std = st_pool.tile([1, R], F32, tag="ln_std")
                nc.scalar.activation(std[:], var[:], AF.Sqrt, bias=epst[:])
                nc.vector.reciprocal(st2[:, 0, :], std[:])
                nmu = st_pool.tile([1, R], F32, tag="ln_nmu")
                nc.vector.tensor_mul(nmu[:], mu[:], st2[:, 0, :])
                nc.vector.tensor_scalar_mul(st2[:, 1, :], nmu[:], -1.0)
                bc = psB.tile([P, 2, R], F32, name="ln_bc", tag="bc")
                nc.tensor.matmul(bc[:].rearrange("p a r -> p (a r)"),
                                 ones1_f[:], st2[:].rearrange("o a r -> o (a r)"),
                                 start=True, stop=True)
                for k in range(KT):
                    t1 = tmp_pool.tile([P, R], F32, tag="ln_t1")
                    nc.vector.tensor_mul(t1[:], xm[:, k], bc[:, 0, :])
                    nc.vector.tensor_add(xm[:, k], t1[:], bc[:, 1, :])
                    nc.vector.tensor_scalar(xm[:, k], xm[:, k],
                                            gs[:, k:k + 1], bes[:, k:k + 1],
                                            ALU.mult, ALU.add)
                    nc.vector.tensor_copy(out=xb[:, k], in_=xm[:, k])

            # ---------------- layers ----------------
            for l in range(n_layers):
                kvin = dram.tile([2, KSZ], BF16, tag="kvin")
                kvout = dram.tile([NG, 2, KSZ], BF16, tag="kvout")
                kview = kvin[0].rearrange("(f r) -> f r", r=R)
                vview = kvin[1].rearrange("(r f) -> r f", f=D)

                # ---- k projection (qkv out-tiles 8..15), transposed ----
                for m in range(KT, 2 * KT):
                    wqm = wq_pool.tile([P, KT, P], BF16, tag="wqm")
                    nc.sync.dma_start(wqm[:], wqkvR[m].rearrange("p (k j) -> p k j", k=KT))
                    ps = psA.tile([P, R], F32, tag="psA")
                    for k in range(KT):
                        nc.tensor.matmul(ps[:], wqm[:, k], xb[:, k],
                                         start=(k == 0), stop=(k == KT - 1))
                    kt = bt_pool.tile([P, R], BF16, tag="ktile")
                    nc.scalar.activation(kt[:], ps[:], AF.Identity, bias=bqkvs[:, m:m + 1])
                    nc.sync.dma_start(kview[(m - KT) * P:(m - KT + 1) * P, :], kt[:])

                # ---- v projection (row-major, 512-wide halves) ----
                for fc in range(2):
                    psv = [psV.tile([KW, 512], F32, name=f"psv{rc}", tag="psv")
                           for rc in range(2)]
                    for k in range(KT):
                        wvk = wv_pool.tile([P, 512], BF16, tag="wvk")
                        nc.sync.dma_start(wvk[:], wvK[k][:, fc * 512:(fc + 1) * 512])
                        for rc in range(2):
                            nc.tensor.matmul(
                                psv[rc][:], xb[:, k, rc * KW:(rc + 1) * KW], wvk[:],
                                start=(k == 0), stop=(k == KT - 1))
                    for rc in range(2):
                        vt = bt_pool.tile([KW, 512], BF16, tag="vtile")
                        nc.vector.tensor_add(vt[:], psv[rc][:],
                                             bvFs[:, fc * 512:(fc + 1) * 512])
                        nc.sync.dma_start(
                            vview[rc * KW:(rc + 1) * KW, fc * 512:(fc + 1) * 512], vt[:])

                # ---- kv all-gather within each 4-core group ----
                nc.gpsimd.collective_compute(
                    "AllGather", ALU.bypass, replica_groups=groups4,
                    ins=[kvin[:]], outs=[kvout[:]])

                # ---- q projection (overlaps the all-gather) ----
                for m in range(KT):
                    wqm = wq_pool.tile([P, KT, P], BF16, tag="wqm")
                    nc.sync.dma_start(wqm[:], wqkvR[m].rearrange("p (k j) -> p k j", k=KT))
                    ps = psA.tile([P, R], F32, tag="psA")
                    for k in range(KT):
                        nc.tensor.matmul(ps[:], wqm[:, k], xb[:, k],
                                         start=(k == 0), stop=(k == KT - 1))
                    nc.scalar.activation(qT2[:, 2 * m], ps[:DH, :], AF.Identity,
                                         bias=bqkvq[:, 2 * m:2 * m + 1])
                    nc.scalar.activation(qT2[:, 2 * m + 1], ps[DH:, :], AF.Identity,
                                         bias=bqkvq[:, 2 * m + 1:2 * m + 2])

                # ---- attention ----
                for h in range(H):
                    kh = kh_pool.tile([DH, NG, R], BF16, tag="kh")
                    for cr in range(NG):
                        src = kvout[cr, 0].rearrange("(f r) -> f r", r=R)
                        nc.sync.dma_start(kh[:, cr], src[h * DH:(h + 1) * DH, :])
                    khf = kh[:].rearrange("d c r -> d (c r)")
                    ov = psO.tile([DH + 1, R], F32, tag="ov")
                    for kc in range(KC):
                        vp = vp_pool.tile([KW, DH + 1], BF16, tag="vp")
                        vsrc = kvout[kc // 2, 1].rearrange("(r f) -> r f", f=D)
                        nc.sync.dma_start(
                            vp[:, :DH],
                            vsrc[(kc % 2) * KW:(kc % 2 + 1) * KW, h * DH:(h + 1) * DH])
                        nc.vector.memset(vp[:, DH:DH + 1], 1.0)
                        sp = psA.tile([KW, R], F32, tag="psA")
                        nc.tensor.matmul(sp[:], khf[:, kc * KW:(kc + 1) * KW],
                                         qT2[:, h], start=True, stop=True)
                        pt = pt_pool.tile([KW, R], BF16, tag="pt")
                        nc.scalar.activation(pt[:], sp[:], AF.Exp, scale=0.125)
                        nc.vector.tensor_mul(pt[:], pt[:], emk[:, kc])
                        nc.tensor.matmul(ov[:], vp[:], pt[:],
                                         start=(kc == 0), stop=(kc == KC - 1))
                    rec = at_pool.tile([1, R], F32, tag="at_rec")
                    nc.vector.reciprocal(rec[:], ov[DH:DH + 1, :])
                    recb = at_pool.tile([1, R], BF16, tag="at_recb")
                    nc.vector.tensor_copy(out=recb[:], in_=rec[:])
                    bcp = psB.tile([DH, R], F32, tag="bc")
                    nc.tensor.matmul(bcp[:], ones1_bf[:], recb[:], start=True, stop=True)
                    bcs = sb_pool.tile([DH, R], F32, tag="at_bcs")
                    nc.scalar.activation(bcs[:], bcp[:], AF.Copy)
                    nc.vector.tensor_mul(oT[(h % 2) * DH:(h % 2 + 1) * DH, h // 2, :],
                                         ov[:DH, :], bcs[:])

                # ---- out-projection + residual ----
                for m in range(KT):
                    ps = psA.tile([P, R], F32, tag="psA")
                    for k in range(KT):
                        nc.tensor.matmul(ps[:], wos[:, k, m * P:(m + 1) * P], oT[:, k],
                                         start=(k == 0), stop=(k == KT - 1))
                    t = tmp_pool.tile([P, R], F32, tag="res_t")
                    nc.scalar.activation(t[:], ps[:], AF.Identity, bias=bos[:, m:m + 1])
                    nc.vector.tensor_add(xm[:, m], xm[:, m], t[:])
                layernorm(g1s, be1s)

                # ---- FFN ----
                for j in range(FF // P):
                    w1j = w1_pool.tile([P, KT, P], BF16, tag="w1j")
                    nc.sync.dma_start(w1j[:], w1R[j].rearrange("p (k i) -> p k i", k=KT))
                    ps = psA.tile([P, R], F32, tag="psA")
                    for k in range(KT):
                        nc.tensor.matmul(ps[:], w1j[:, k], xb[:, k],
                                         start=(k == 0), stop=(k == KT - 1))
                    if os.environ.get("KBENCH_GELU") == "tanh":
                        # sim-checkable tanh-gelu composite (CoreSim lacks Gelu)
                        xg = tmp_pool.tile([P, R], F32, tag="gl_x")
                        nc.vector.tensor_scalar(xg[:], ps[:], b1s[:, j:j + 1], None, ALU.add)
                        x3 = tmp_pool.tile([P, R], F32, tag="gl_x3")
                        nc.vector.tensor_mul(x3[:], xg[:], xg[:])
                        nc.vector.tensor_mul(x3[:], x3[:], xg[:])
                        nc.vector.tensor_scalar(x3[:], x3[:], 0.044715, None, ALU.mult)
                        nc.vector.tensor_add(x3[:], x3[:], xg[:])
                        th = tmp_pool.tile([P, R], F32, tag="gl_th")
                        nc.scalar.activation(th[:], x3[:], AF.Tanh, scale=0.7978845608)
                        nc.vector.tensor_scalar(th[:], th[:], 1.0, 0.5, ALU.add, ALU.mult)
                        nc.vector.tensor_mul(h32[:, j], xg[:], th[:])
                    else:
                        nc.scalar.activation(h32[:, j], ps[:], AF.Gelu, bias=b1s[:, j:j + 1])
                for m in range(KT):
                    ps = psA.tile([P, R], F32, tag="psA")
                    for j in range(FF // P):
                        nc.tensor.matmul(ps[:], w2s[:, j, m * P:(m + 1) * P], h32[:, j],
                                         start=(j == 0), stop=(j == FF // P - 1))
                    t = tmp_pool.tile([P, R], F32, tag="res_t")
                    nc.scalar.activation(t[:], ps[:], AF.Identity, bias=b2s[:, m:m + 1])
                    nc.vector.tensor_add(xm[:, m], xm[:, m], t[:])
                layernorm(g2s, be2s)

            # ---------------- final logits + CE ----------------
            ps_lse = psT.tile([1, R], F32, name="ce_lse", tag="stat_a")
            ps_tgt = psT.tile([1, R], F32, name="ce_tgt", tag="stat_b")
            for m in range(V // P):
                emm = wq_pool.tile([P, KT, P], BF16, tag="wqm")
                nc.sync.dma_start(emm[:], embR[m].rearrange("p (k j) -> p k j", k=KT))
                ps = psA.tile([P, R], F32, tag="psA")
                for k in range(KT):
                    nc.tensor.matmul(ps[:], emm[:, k], xb[:, k],
                                     start=(k == 0), stop=(k == KT - 1))
                ex = tmp_pool.tile([P, R], F32, tag="ce_ex")
                nc.scalar.activation(ex[:], ps[:], AF.Exp)
                nc.tensor.matmul(ps_lse[:], ones_f[:], ex[:],
                                 start=(m == 0), stop=(m == V // P - 1))
                tg = tmp_pool.tile([P, R], F32, tag="ce_tg")
                nc.vector.tensor_mul(tg[:], ps[:], ohs[:, m])
                nc.tensor.matmul(ps_tgt[:], ones_f[:], tg[:],
                                 start=(m == 0), stop=(m == V // P - 1))
            lse = st_pool.tile([1, R], F32, name="ce_lse_s")
            nc.scalar.activation(lse[:], ps_lse[:], AF.Ln)
            nll = st_pool.tile([1, R], F32, name="ce_nll")
            nc.vector.tensor_sub(nll[:], lse[:], ps_tgt[:])
            nc.vector.tensor_mul(nll[:], nll[:], wvs[:])
            cet = st_pool.tile([1, 2], F32, name="ce_pair")
            nc.vector.reduce_sum(cet[:, 0:1], nll[:], axis=AX.X)
            nc.vector.reduce_sum(cet[:, 1:2], wvs[:], axis=AX.X)
            ce_in = dram1.tile([1, 2], F32, name="ce_in")
            ce_out = dram1.tile([1, 2], F32, name="ce_out", addr_space="Shared")
            nc.sync.dma_start(ce_in[:], cet[:])
            nc.gpsimd.collective_compute(
                "AllReduce", ALU.add, replica_groups=groups8,
                ins=[ce_in[:]], outs=[ce_out[:]])
            ces = st_pool.tile([1, 2], F32, name="ce_sum")
            nc.sync.dma_start(ces[:], ce_out[:])
            den = st_pool.tile([1, 1], F32, name="ce_den")
            nc.vector.tensor_scalar_max(den[:], ces[:, 1:2], 1.0)
            rden = st_pool.tile([1, 1], F32, name="ce_rden")
            nc.vector.reciprocal(rden[:], den[:])
            lossv = st_pool.tile([1, 1], F32, name="ce_loss")
            nc.vector.tensor_mul(lossv[:], ces[:, 0:1], rden[:])
            nc.sync.dma_start(loss_out[:], lossv[:])

    nc.compile()
    _PROG_CACHE[n_layers] = nc
    return nc


def kernel(**inputs):
    global LAST_EXEC_NS, LAST_RESULTS
    n_layers = int(os.environ.get("KBENCH_LAYERS", NL_DEFAULT))
    nc = build_program(n_layers)
    in_maps = prepare_inputs(**inputs)
    trace = bool(int(os.environ.get("KBENCH_TRACE", "0")))
    res = run_bass_kernel_spmd(nc, in_maps, list(range(8)), trace=trace)
    LAST_EXEC_NS = res.exec_time_ns
    LAST_RESULTS = res
    loss = np.asarray(res.results[0]["loss"]).reshape(())
    return np.float32(loss)


if __name__ == "__main__":
    build_program(int(os.environ.get("KBENCH_LAYERS", NL_DEFAULT)))
    print("program built OK")
